# revision 1
# baseline (speedup 1.0000x reference)
import numpy as np
from contextlib import ExitStack

import concourse.mybir as mybir
import concourse.bass as bass
import concourse.tile as tile
from concourse.bass_utils import run_bass_kernel_spmd

# Problem: nn_Predictor (moe_routing). L=6 streams, B=16384, D=512, NC=3992, 4 experts.
# Sharding: pure data parallel over B across 8 cores; weights replicated.
L, B, D, NCLS, NE = 6, 16384, 512, 3992, 4
NCORES = 8
BS = B // NCORES            # 2048 tokens per core
TT = 512                    # token tile
NTILES = BS // TT           # 4
NSUB = TT // 128            # 4 token subtiles per tile
KC = 24                     # 128-wide K chunks of flat (6*512/128)
NCH = (NCLS + 511) // 512   # 8 output column chunks (last = 408)

F32 = mybir.dt.float32
F32R = mybir.dt.float32r


def _r(ap):
    return ap.bitcast(F32R)


def _build():
    nc = bass.Bass("TRN2")

    fusion = nc.dram_tensor("fusion", [L, BS, D], F32, kind="ExternalInput")
    masksT = nc.dram_tensor("masksT", [NE, BS], F32, kind="ExternalInput")
    w1_in = [
        nc.dram_tensor("w1_0", [1536, 512], F32, kind="ExternalInput"),
        nc.dram_tensor("w1_1", [1536, 512], F32, kind="ExternalInput"),
        nc.dram_tensor("w1_2", [3072, 512], F32, kind="ExternalInput"),
        nc.dram_tensor("w1_3", [3072, 512], F32, kind="ExternalInput"),
    ]
    b1all = nc.dram_tensor("b1all", [NE * 512], F32, kind="ExternalInput")
    w2all = nc.dram_tensor("w2all", [NE, 512, 512], F32, kind="ExternalInput")
    b2s = nc.dram_tensor("b2s", [1, NE * 512], F32, kind="ExternalInput")
    dw1 = nc.dram_tensor("dw1", [512, 512], F32, kind="ExternalInput")
    db1 = nc.dram_tensor("db1", [512], F32, kind="ExternalInput")
    dw2 = nc.dram_tensor("dw2", [512, NCLS], F32, kind="ExternalInput")
    db2 = nc.dram_tensor("db2", [1, NCLS], F32, kind="ExternalInput")
    identD = nc.dram_tensor("ident128", [128, 128], F32, kind="ExternalInput")
    out = nc.dram_tensor("out", [BS, NCLS], F32, kind="ExternalOutput")

    # M-tile table for the W1 stage: (expert, flatT chunk range)
    # e0 eats front (chunks 0..11), e1 back (12..23), e2/e3 all 24.
    # e3's input scaling (a on front, b on back) is folded into w1_3 on host.
    w1_mtiles = []
    for e, (klo, nk) in enumerate([(0, 12), (12, 12), (0, 24), (0, 24)]):
        for mloc in range(4):
            w1_mtiles.append((e, mloc, klo, nk))

    with tile.TileContext(nc) as tc, ExitStack() as ctx:
        singles = ctx.enter_context(tc.tile_pool(name="singles", bufs=1))
        natP = ctx.enter_context(tc.tile_pool(name="natP", bufs=3))
        flatP = ctx.enter_context(tc.tile_pool(name="flatP", bufs=KC + 1))
        w1P = ctx.enter_context(tc.tile_pool(name="w1P", bufs=2))
        htP = ctx.enter_context(tc.tile_pool(name="htP", bufs=3))
        mbP = ctx.enter_context(tc.tile_pool(name="mbP", bufs=5))
        selP = ctx.enter_context(tc.tile_pool(name="selP", bufs=4))
        sigP = ctx.enter_context(tc.tile_pool(name="sigP", bufs=5))
        dw2P = ctx.enter_context(tc.tile_pool(name="dw2P", bufs=2))
        outP = ctx.enter_context(tc.tile_pool(name="outP", bufs=2))
        db2bcP = ctx.enter_context(tc.tile_pool(name="db2bcP", bufs=2))

        tposePs = ctx.enter_context(tc.tile_pool(name="tposePs", bufs=1, space="PSUM"))
        w1Ps = ctx.enter_context(tc.tile_pool(name="w1Ps", bufs=1, space="PSUM"))
        w2Ps = ctx.enter_context(tc.tile_pool(name="w2Ps", bufs=4, space="PSUM"))
        d2Ps = ctx.enter_context(tc.tile_pool(name="d2Ps", bufs=2, space="PSUM"))

        # identity via DMA (not Pool) so transposes carry a single coalesced
        # DMA-semaphore wait: walrus fits only one sync wait on the LW struct.
        ident = singles.tile([128, 128], F32R)
        nc.sync.dma_start(out=ident, in_=_r(identD[:, :]))

        # biases
        b1sb = singles.tile([128, 16], F32)     # [:, mi] = b1 of W1-stage M-tile mi
        nc.sync.dma_start(
            out=b1sb, in_=bass.AP(tensor=b1all, offset=0, ap=[[1, 128], [128, 16]])
        )
        b2sb = singles.tile([1, NE * 512], F32)
        nc.sync.dma_start(out=_r(b2sb), in_=_r(b2s[:, :]))
        db1sb = singles.tile([128, 4], F32)
        nc.sync.dma_start(
            out=db1sb, in_=bass.AP(tensor=db1, offset=0, ap=[[1, 128], [128, 4]])
        )

        # resident weights: W2 (lhsT layout) and dec_W1 (lhsT layout)
        w2sb = []
        for e in range(NE):
            w2e = singles.tile([128, 4 * 512], F32, name=f"w2sb{e}")
            nc.sync.dma_start(
                out=_r(w2e),
                in_=_r(bass.AP(
                    tensor=w2all,
                    offset=e * 512 * 512,
                    ap=[[512, 128], [128 * 512, 4], [1, 512]],
                )),
            )
            w2sb.append(w2e)
        dw1sb = singles.tile([128, 4 * 512], F32)
        nc.sync.dma_start(
            out=_r(dw1sb),
            in_=_r(bass.AP(tensor=dw1, offset=0, ap=[[512, 128], [128 * 512, 4], [1, 512]])),
        )

        for it in range(NTILES):
            t0 = it * TT

            # ---- stage A: load + transpose -> flatT chunks [128 feat, 512 tok]
            flatT = []
            for c in range(KC):
                l, off = c // 4, (c % 4) * 128
                natc = natP.tile([128, NSUB, 128], F32, name="natc")
                nc.sync.dma_start(
                    out=_r(natc),
                    in_=_r(bass.AP(
                        tensor=fusion,
                        offset=l * BS * D + t0 * D + off,
                        ap=[[D, 128], [128 * D, NSUB], [1, 128]],
                    )),
                )
                pT = tposePs.tile([128, TT], F32, name="pT")
                for s in range(NSUB):
                    nc.tensor.transpose(
                        _r(pT[:, s * 128 : (s + 1) * 128]), _r(natc[:, s, :]), _r(ident)
                    )
                fc = flatP.tile([128, TT], F32, name="fc")
                nc.any.tensor_copy(out=_r(fc), in_=pT)
                flatT.append(fc)

            # ---- broadcast one-hot expert masks [128, TT] per expert
            maskB = []
            for e in range(NE):
                mb = mbP.tile([128, TT], F32, name="mb")
                nc.sync.dma_start(
                    out=_r(mb),
                    in_=_r(bass.AP(
                        tensor=masksT, offset=e * BS + t0, ap=[[0, 128], [1, TT]]
                    )),
                )
                maskB.append(mb)

            # ---- stage B+C fused: W1 + bias + relu + mask, each ht chunk
            # immediately accumulated into the 4 selT psum banks via W2.
            w2ps = [w2Ps.tile([128, TT], F32, name="w2ps") for _ in range(4)]
            for mi, (e, mloc, klo, nk) in enumerate(w1_mtiles):
                ps = w1Ps.tile([128, TT], F32, name="w1ps")
                ki = 0
                for kb in range(0, nk, 12):
                    nb = min(12, nk - kb)
                    w1t = w1P.tile([128, nb * 128], F32, name="w1t")
                    nc.sync.dma_start(
                        out=_r(w1t),
                        in_=_r(bass.AP(
                            tensor=w1_in[e],
                            offset=(kb * 512 * 128) + mloc * 128,
                            ap=[[512, 128], [128 * 512, nb], [1, 128]],
                        )),
                    )
                    for kj in range(nb):
                        nc.tensor.matmul(
                            ps,
                            _r(w1t[:, kj * 128 : (kj + 1) * 128]),
                            _r(flatT[klo + ki]),
                            start=(ki == 0),
                            stop=(ki == nk - 1),
                        )
                        ki += 1
                h = htP.tile([128, TT], F32, name="h")
                nc.scalar.activation(
                    _r(h), ps, mybir.ActivationFunctionType.Relu,
                    bias=b1sb[:, mi : mi + 1], scale=1.0,
                )
                nc.vector.tensor_tensor(
                    out=_r(h), in0=h, in1=maskB[e], op=mybir.AluOpType.mult
                )
                for md in range(4):
                    nc.tensor.matmul(
                        w2ps[md],
                        _r(w2sb[e][:, mloc * 512 + md * 128 : mloc * 512 + md * 128 + 128]),
                        _r(h),
                        start=(mi == 0),
                        stop=False,
                    )

            # selected-expert W2 bias, then copy selT out of PSUM
            selT = []
            for md in range(4):
                for e in range(NE):
                    nc.tensor.matmul(
                        w2ps[md],
                        _r(b2sb[0:1, e * 512 + md * 128 : e * 512 + (md + 1) * 128]),
                        _r(maskB[e][0:1, :]),
                        start=False,
                        stop=(e == NE - 1),
                    )
                st = selP.tile([128, TT], F32, name="st")
                nc.any.tensor_copy(out=_r(st), in_=w2ps[md])
                selT.append(st)

            # ---- stage D: dec1 + sigmoid -> sigT [4][128 h2, TT]
            sigT = []
            for mh in range(4):
                ps = w1Ps.tile([128, TT], F32, name="w1ps")
                for kd in range(4):
                    nc.tensor.matmul(
                        ps,
                        _r(dw1sb[:, kd * 512 + mh * 128 : kd * 512 + mh * 128 + 128]),
                        _r(selT[kd]),
                        start=(kd == 0),
                        stop=(kd == 3),
                    )
                sg = sigP.tile([128, TT], F32, name="sg")
                nc.scalar.activation(
                    _r(sg), ps, mybir.ActivationFunctionType.Sigmoid,
                    bias=db1sb[:, mh : mh + 1], scale=1.0,
                )
                sigT.append(sg)

            # ---- stage E: dec2 (flip to natural) + bias -> out
            for n in range(NCH):
                nw = min(512, NCLS - n * 512)
                dwt = dw2P.tile([128, 4, nw], F32, name="dwt")
                nc.sync.dma_start(
                    out=_r(dwt),
                    in_=_r(bass.AP(
                        tensor=dw2,
                        offset=n * 512,
                        ap=[[NCLS, 128], [128 * NCLS, 4], [1, nw]],
                    )),
                )
                db2bc = db2bcP.tile([128, nw], F32, name="db2bc")
                nc.sync.dma_start(
                    out=db2bc,
                    in_=bass.AP(tensor=db2, offset=n * 512, ap=[[0, 128], [1, nw]]),
                )
                for s in range(NSUB):
                    ps = d2Ps.tile([128, 512], F32, name="d2ps")
                    for kh in range(4):
                        nc.tensor.matmul(
                            ps[:, :nw],
                            _r(sigT[kh][:, s * 128 : (s + 1) * 128]),
                            _r(dwt[:, kh, :]),
                            start=(kh == 0),
                            stop=(kh == 3),
                        )
                    ot = outP.tile([128, 512], F32, name="ot")
                    nc.vector.tensor_tensor(
                        out=ot[:, :nw], in0=ps[:, :nw], in1=db2bc,
                        op=mybir.AluOpType.add,
                    )
                    nc.sync.dma_start(
                        out=out[t0 + s * 128 : t0 + (s + 1) * 128, n * 512 : n * 512 + nw],
                        in_=ot[:, :nw],
                    )
    # walrus allows at most 1 sync wait per Matmult; split extras into
    # EventSemaphore instructions (same pass Bacc.compile runs)
    import bass_rust

    bass_rust.generate_event_semaphores(nc)
    return nc


_NC_CACHE = None


def _get_nc():
    global _NC_CACHE
    if _NC_CACHE is None:
        _NC_CACHE = _build()
    return _NC_CACHE


def _prep_inputs(inputs):
    f32 = np.float32
    x = np.asarray(inputs["fusion_hs"], f32)                      # [L, B, D]
    flat = np.transpose(x, (1, 0, 2)).reshape(B, L * D)

    logits = flat.astype(np.float64) @ np.asarray(inputs["gate_W"], f32).astype(
        np.float64
    ) + np.asarray(inputs["gate_b"], f32).astype(np.float64)
    am = np.argmax(logits, axis=1)
    masksT = np.zeros((NE, B), f32)
    masksT[am, np.arange(B)] = 1.0

    w1_3s = np.array(inputs["e3_W1"], f32, copy=True)
    w1_3s[: 3 * D] *= f32(np.asarray(inputs["e3_a"]).reshape(-1)[0])
    w1_3s[3 * D :] *= f32(np.asarray(inputs["e3_b"]).reshape(-1)[0])

    common = {
        "w1_0": np.ascontiguousarray(inputs["e0_W1"], f32),
        "w1_1": np.ascontiguousarray(inputs["e1_W1"], f32),
        "w1_2": np.ascontiguousarray(inputs["e2_W1"], f32),
        "w1_3": np.ascontiguousarray(w1_3s),
        "b1all": np.concatenate(
            [np.asarray(inputs[f"e{e}_b1"], f32) for e in range(NE)]
        ),
        "w2all": np.ascontiguousarray(
            np.stack([np.asarray(inputs[f"e{e}_W2"], f32) for e in range(NE)])
        ),
        "b2s": np.concatenate(
            [np.asarray(inputs[f"e{e}_b2"], f32) for e in range(NE)]
        ).reshape(1, NE * 512),
        "dw1": np.ascontiguousarray(inputs["dec_W1"], f32),
        "db1": np.ascontiguousarray(inputs["dec_b1"], f32),
        "dw2": np.ascontiguousarray(inputs["dec_W2"], f32),
        "db2": np.ascontiguousarray(
            np.asarray(inputs["dec_b2"], f32).reshape(1, NCLS)
        ),
        "ident128": np.eye(128, dtype=f32),
    }
    in_maps = []
    for c in range(NCORES):
        sl = slice(c * BS, (c + 1) * BS)
        m = dict(common)
        m["fusion"] = np.ascontiguousarray(x[:, sl, :])
        m["masksT"] = np.ascontiguousarray(masksT[:, sl])
        in_maps.append(m)
    return in_maps


def kernel(**inputs):
    nc = _get_nc()
    in_maps = _prep_inputs(inputs)
    res = run_bass_kernel_spmd(nc, in_maps, core_ids=list(range(NCORES)))
    return np.concatenate([res.results[c]["out"] for c in range(NCORES)], axis=0)



# revision 4
# speedup vs baseline: 3.6665x; 3.6665x over previous
import numpy as np
import ml_dtypes
from contextlib import ExitStack

import concourse.mybir as mybir
import concourse.bass as bass
import concourse.tile as tile
from concourse.bass_utils import run_bass_kernel_spmd

# Problem: nn_Predictor (moe_routing). L=6 streams, B=16384, D=512, NC=3992, 4 experts.
# Strategy: host computes the hard gate (argmax of logits, fp64) and expert-sorts
# the tokens; each core gets floor(N_e/8) tokens of each expert (identical segment
# layout across cores -> one SPMD program), leftover <=28 tokens are evaluated on
# the host in numpy. Each token only runs through its own expert (4x less W1/W2
# work than dense). Inputs are pre-transposed to feature-major bf16 on the host so
# the device does zero transposes; all weights are bf16, swizzled host-side into
# their exact SBUF layouts and resident for the whole kernel.
L, B, D, NCLS, NE = 6, 16384, 512, 3992, 4
NCORES = 8
NDEV = 2048                 # device tokens per core (4 tiles of 512)
TT = 512
NT = NDEV // TT
NK = [12, 12, 24, 24]       # 128-wide K chunks of W1 per expert
KLO = [0, 12, 0, 0]         # first flatT chunk each expert reads
NCH = (NCLS + 511) // 512   # 8 output column chunks (last = 408)

F32 = mybir.dt.float32
BF16 = mybir.dt.bfloat16
bf16 = ml_dtypes.bfloat16
AF = mybir.ActivationFunctionType


def _build(bounds):
    """bounds: cumulative token boundaries (b0,b1,b2,2048) of the 4 expert
    segments in each core's 2048-token stream (same for every core)."""
    nc = bass.Bass("TRN2")

    fusionT = nc.dram_tensor("fusionT", [L * D, NDEV], BF16, kind="ExternalInput")
    w1s = [
        nc.dram_tensor(f"w1s{e}", [128, NK[e] * 512], BF16, kind="ExternalInput")
        for e in range(NE)
    ]
    w2s = nc.dram_tensor("w2s", [128, NE * 2048], BF16, kind="ExternalInput")
    dw1s = nc.dram_tensor("dw1s", [128, 2048], BF16, kind="ExternalInput")
    dw2s = nc.dram_tensor("dw2s", [128, 4 * NCLS], BF16, kind="ExternalInput")
    b1b = nc.dram_tensor("b1b", [128, NE * 4], F32, kind="ExternalInput")
    b2b = nc.dram_tensor("b2b", [128, NE * 4], F32, kind="ExternalInput")
    db1b = nc.dram_tensor("db1b", [128, 4], F32, kind="ExternalInput")
    out = nc.dram_tensor("out", [NDEV, NCLS], BF16, kind="ExternalOutput")

    # global segment list (expert, lo, hi), then intersect with each 512 tile
    segs = []
    prev = 0
    for e, b in enumerate(bounds):
        if b > prev:
            segs.append((e, prev, b))
            prev = b
    tile_segs = []
    for t in range(NT):
        t0, t1 = t * TT, (t + 1) * TT
        tile_segs.append(
            [(e, max(lo, t0) - t0, min(hi, t1) - t0) for (e, lo, hi) in segs
             if lo < t1 and hi > t0]
        )

    with tile.TileContext(nc) as tc, ExitStack() as ctx:
        singles = ctx.enter_context(tc.tile_pool(name="singles", bufs=1))
        flatP = ctx.enter_context(tc.tile_pool(name="flatP", bufs=36))
        htP = ctx.enter_context(tc.tile_pool(name="htP", bufs=6))
        selP = ctx.enter_context(tc.tile_pool(name="selP", bufs=6))
        sigP = ctx.enter_context(tc.tile_pool(name="sigP", bufs=6))
        outP = ctx.enter_context(tc.tile_pool(name="outP", bufs=6))

        psA = ctx.enter_context(tc.tile_pool(name="psA", bufs=2, space="PSUM"))
        psB = ctx.enter_context(tc.tile_pool(name="psB", bufs=4, space="PSUM"))
        psC = ctx.enter_context(tc.tile_pool(name="psC", bufs=2, space="PSUM"))

        flat_tiles = [None] * NT

        def load_tile(t):
            need = sorted({c for (e, _, _) in tile_segs[t]
                           for c in range(KLO[e], KLO[e] + NK[e])})
            d = {}
            for c in need:
                fc = flatP.tile([128, TT], BF16, name="fc")
                nc.sync.dma_start(
                    out=fc,
                    in_=bass.AP(tensor=fusionT, offset=c * 128 * NDEV + t * TT,
                                ap=[[NDEV, 128], [1, TT]]),
                )
                d[c] = fc
            flat_tiles[t] = d

        # tile-0 activations first so compute starts ASAP, then weights in
        # order of first use; everything streams behind compute.
        load_tile(0)
        first_e = tile_segs[0][0][0]
        w1sb = [None] * NE
        w1sb[first_e] = singles.tile([128, NK[first_e] * 512], BF16, name=f"w1sb{first_e}")
        nc.sync.dma_start(out=w1sb[first_e], in_=w1s[first_e][:, :])
        b1sb = singles.tile([128, NE * 4], F32)
        nc.sync.dma_start(out=b1sb, in_=b1b[:, :])
        b2sb = singles.tile([128, NE * 4], F32)
        nc.sync.dma_start(out=b2sb, in_=b2b[:, :])
        db1sb = singles.tile([128, 4], F32)
        nc.sync.dma_start(out=db1sb, in_=db1b[:, :])
        w2sb = singles.tile([128, NE * 2048], BF16)
        nc.sync.dma_start(out=w2sb, in_=w2s[:, :])
        dw1sb = singles.tile([128, 2048], BF16)
        nc.sync.dma_start(out=dw1sb, in_=dw1s[:, :])
        dw2sb = singles.tile([128, 4 * NCLS], BF16)
        nc.sync.dma_start(out=dw2sb, in_=dw2s[:, :])
        for e in range(NE):
            if w1sb[e] is None:
                w1sb[e] = singles.tile([128, NK[e] * 512], BF16, name=f"w1sb{e}")
                nc.sync.dma_start(out=w1sb[e], in_=w1s[e][:, :])

        sig_tiles = [None] * NT

        def emit_front(t):
            """W1 -> relu -> W2 -> (+b2) -> selT for tile t."""
            ft = flat_tiles[t]
            selB = [psB.tile([128, TT], F32, name="selps") for _ in range(4)]
            nseg = len(tile_segs[t])
            for si, (e, lo, hi) in enumerate(tile_segs[t]):
                w = hi - lo
                for m in range(4):
                    hps = psA.tile([128, TT], F32, name="hps")
                    for ki in range(NK[e]):
                        nc.tensor.matmul(
                            hps[:, :w],
                            w1sb[e][:, (ki * 4 + m) * 128: (ki * 4 + m + 1) * 128],
                            ft[KLO[e] + ki][:, lo:hi],
                            start=(ki == 0),
                            stop=(ki == NK[e] - 1),
                        )
                    ht = htP.tile([128, TT], BF16, name="ht")
                    nc.scalar.activation(
                        ht[:, :w], hps[:, :w], AF.Relu,
                        bias=b1sb[:, e * 4 + m: e * 4 + m + 1], scale=1.0,
                    )
                    for md in range(4):
                        nc.tensor.matmul(
                            selB[md][:, lo:hi],
                            w2sb[:, e * 2048 + m * 512 + md * 128:
                                 e * 2048 + m * 512 + (md + 1) * 128],
                            ht[:, :w],
                            start=(m == 0),
                            stop=(m == 3),
                        )
            selT = []
            for md in range(4):
                st = selP.tile([128, TT], BF16, name="st")
                for (e, lo, hi) in tile_segs[t]:
                    nc.scalar.activation(
                        st[:, lo:hi], selB[md][:, lo:hi], AF.Identity,
                        bias=b2sb[:, e * 4 + md: e * 4 + md + 1], scale=1.0,
                    )
                selT.append(st)
            return selT

        def emit_dec1(t, selT):
            sigT = []
            for mh in range(4):
                ps = psA.tile([128, TT], F32, name="hps")
                for kd in range(4):
                    nc.tensor.matmul(
                        ps,
                        dw1sb[:, kd * 512 + mh * 128: kd * 512 + (mh + 1) * 128],
                        selT[kd],
                        start=(kd == 0),
                        stop=(kd == 3),
                    )
                sg = sigP.tile([128, TT], BF16, name="sg")
                nc.scalar.activation(
                    sg, ps, AF.Sigmoid, bias=db1sb[:, mh: mh + 1], scale=1.0,
                )
                sigT.append(sg)
            sig_tiles[t] = sigT

        def emit_dec2(t):
            sigT = sig_tiles[t]
            for n in range(NCH):
                nw = min(512, NCLS - n * 512)
                for s in range(TT // 128):
                    ps = psC.tile([128, 512], F32, name="d2ps")
                    for kh in range(4):
                        nc.tensor.matmul(
                            ps[:, :nw],
                            sigT[kh][:, s * 128: (s + 1) * 128],
                            dw2sb[:, kh * NCLS + n * 512: kh * NCLS + n * 512 + nw],
                            start=(kh == 0),
                            stop=(kh == 3),
                        )
                    ot = outP.tile([128, 512], BF16, name="ot")
                    nc.vector.tensor_copy(out=ot[:, :nw], in_=ps[:, :nw])
                    nc.sync.dma_start(
                        out=out[t * TT + s * 128: t * TT + (s + 1) * 128,
                                n * 512: n * 512 + nw],
                        in_=ot[:, :nw],
                    )

        # software pipeline: dec2 of tile t-1 (a long PE stretch with no new
        # ACT dependencies) is emitted between W2(t) and dec1(t) so the PE
        # never waits on the ACT copies feeding dec1/dec2.
        for t in range(NT):
            if t + 1 < NT:
                load_tile(t + 1)
            selT = emit_front(t)
            if t > 0:
                emit_dec2(t - 1)
            emit_dec1(t, selT)
        emit_dec2(NT - 1)

    import bass_rust

    bass_rust.generate_event_semaphores(nc)
    return nc


_NC_CACHE = {}


def _get_nc(bounds):
    key = tuple(bounds)
    if key not in _NC_CACHE:
        _NC_CACHE[key] = _build(key)
    return _NC_CACHE[key]


def _swizzle(w):
    """[K, M] f32 -> lhsT SBUF layout [128, (K/128)*(M/128)*128] bf16 where
    col (k*(M/128)+m)*128+j holds w[k*128+p, m*128+j]."""
    K, M = w.shape
    return np.ascontiguousarray(
        w.reshape(K // 128, 128, M // 128, 128).transpose(1, 0, 2, 3)
        .reshape(128, (K // 128) * M).astype(bf16)
    )


def _prep(inputs):
    f32 = np.float32
    x = np.asarray(inputs["fusion_hs"], f32)                 # [L, B, D]
    flat = np.ascontiguousarray(np.transpose(x, (1, 0, 2)).reshape(B, L * D))

    logits = flat.astype(np.float64) @ np.asarray(inputs["gate_W"], f32).astype(
        np.float64
    ) + np.asarray(inputs["gate_b"], f32).astype(np.float64)
    am = np.argmax(logits, axis=1)

    idx = [np.nonzero(am == e)[0] for e in range(NE)]
    ke = [len(idx[e]) // NCORES for e in range(NE)]
    pad = NDEV - sum(ke)
    assert pad >= 0
    bounds = (ke[0], ke[0] + ke[1], ke[0] + ke[1] + ke[2], NDEV)

    w1_3s = np.array(inputs["e3_W1"], f32, copy=True)
    w1_3s[: 3 * D] *= f32(np.asarray(inputs["e3_a"]).reshape(-1)[0])
    w1_3s[3 * D:] *= f32(np.asarray(inputs["e3_b"]).reshape(-1)[0])
    W1 = [np.asarray(inputs["e0_W1"], f32), np.asarray(inputs["e1_W1"], f32),
          np.asarray(inputs["e2_W1"], f32), w1_3s]
    W2 = [np.asarray(inputs[f"e{e}_W2"], f32) for e in range(NE)]
    b1 = [np.asarray(inputs[f"e{e}_b1"], f32) for e in range(NE)]
    b2 = [np.asarray(inputs[f"e{e}_b2"], f32) for e in range(NE)]
    dW1 = np.asarray(inputs["dec_W1"], f32)
    db1 = np.asarray(inputs["dec_b1"], f32)
    dW2 = np.asarray(inputs["dec_W2"], f32)
    db2 = np.asarray(inputs["dec_b2"], f32)

    common = {
        "w1s0": _swizzle(W1[0]), "w1s1": _swizzle(W1[1]),
        "w1s2": _swizzle(W1[2]), "w1s3": _swizzle(W1[3]),
        "w2s": np.concatenate([_swizzle(w) for w in W2], axis=1),
        "dw1s": _swizzle(dW1),
        "dw2s": np.ascontiguousarray(
            dW2.reshape(4, 128, NCLS).transpose(1, 0, 2).reshape(128, 4 * NCLS)
            .astype(bf16)
        ),
        "b1b": np.stack([b1[e][m * 128: (m + 1) * 128]
                         for e in range(NE) for m in range(4)], axis=1),
        "b2b": np.stack([b2[e][m * 128: (m + 1) * 128]
                         for e in range(NE) for m in range(4)], axis=1),
        "db1b": np.stack([db1[m * 128: (m + 1) * 128] for m in range(4)], axis=1),
    }

    perms, in_maps = [], []
    for c in range(NCORES):
        perm = np.concatenate(
            [idx[e][c * ke[e]: (c + 1) * ke[e]] for e in range(NE)]
        )
        perms.append(perm)
        a = flat[perm].astype(bf16)                          # [ndev_real, 3072]
        ft = np.zeros((L * D, NDEV), bf16)
        ft[:, : len(perm)] = a.T
        m = dict(common)
        m["fusionT"] = ft
        in_maps.append(m)

    # leftover tokens (<= 28): full forward on host in fp32
    lt = np.concatenate([idx[e][NCORES * ke[e]:] for e in range(NE)])
    lt_out = np.zeros((len(lt), NCLS), f32)
    if len(lt):
        off = 0
        ins = [flat[:, : 3 * D], flat[:, 3 * D:], flat, flat]
        for e in range(NE):
            g = idx[e][NCORES * ke[e]:]
            if len(g) == 0:
                continue
            h = np.maximum(ins[e][g] @ W1[e] + b1[e], 0)
            sel = h @ W2[e] + b2[e]
            sig = 1.0 / (1.0 + np.exp(-(sel @ dW1 + db1)))
            lt_out[off: off + len(g)] = sig @ dW2 + db2
            off += len(g)

    return bounds, in_maps, perms, lt, lt_out, db2


def kernel(**inputs):
    bounds, in_maps, perms, lt, lt_out, db2 = _prep(inputs)
    nc = _get_nc(bounds)
    res = run_bass_kernel_spmd(nc, in_maps, core_ids=list(range(NCORES)))
    out = np.empty((B, NCLS), np.float32)
    for c in range(NCORES):
        dev = np.asarray(res.results[c]["out"], np.float32)
        out[perms[c]] = dev[: len(perms[c])] + db2
    if len(lt):
        out[lt] = lt_out
    return out


# revision 11
# speedup vs baseline: 3.6946x; 1.0077x over previous
import numpy as np
import ml_dtypes
from contextlib import ExitStack

import concourse.mybir as mybir
import concourse.bass as bass
import concourse.tile as tile
from concourse.bass_utils import run_bass_kernel_spmd

# Problem: nn_Predictor (moe_routing). L=6 streams, B=16384, D=512, NC=3992, 4 experts.
# Strategy: host computes the hard gate (argmax of logits, fp64) and expert-sorts
# the tokens; each core gets floor(N_e/8) tokens of each expert (identical segment
# layout across cores -> one SPMD program), leftover <=28 tokens are evaluated on
# the host in numpy. Each token only runs through its own expert (4x less W1/W2
# work than dense). Inputs are pre-transposed to feature-major bf16 on the host so
# the device does zero transposes; all weights are bf16, swizzled host-side into
# their exact SBUF layouts and resident for the whole kernel.
L, B, D, NCLS, NE = 6, 16384, 512, 3992, 4
NCORES = 8
NDEV = 2048                 # device tokens per core (4 tiles of 512)
TT = 512
NT = NDEV // TT
NK = [12, 12, 24, 24]       # 128-wide K chunks of W1 per expert
KLO = [0, 12, 0, 0]         # first flatT chunk each expert reads
NCH = (NCLS + 511) // 512   # 8 output column chunks (last = 408)

F32 = mybir.dt.float32
BF16 = mybir.dt.bfloat16
bf16 = ml_dtypes.bfloat16
AF = mybir.ActivationFunctionType


def _build(bounds):
    """bounds: cumulative token boundaries (b0,b1,b2,2048) of the 4 expert
    segments in each core's 2048-token stream (same for every core)."""
    nc = bass.Bass("TRN2")

    fusionT = nc.dram_tensor("fusionT", [L * D, NDEV], BF16, kind="ExternalInput")
    w1s = [
        nc.dram_tensor(f"w1s{e}", [128, NK[e] * 512], BF16, kind="ExternalInput")
        for e in range(NE)
    ]
    w2s = nc.dram_tensor("w2s", [128, NE * 2048], BF16, kind="ExternalInput")
    dw1s = nc.dram_tensor("dw1s", [128, 2048], BF16, kind="ExternalInput")
    dw2s = nc.dram_tensor("dw2s", [128, 4 * NCLS], BF16, kind="ExternalInput")
    b1b = nc.dram_tensor("b1b", [128, NE * 4], F32, kind="ExternalInput")
    b2b = nc.dram_tensor("b2b", [128, NE * 4], F32, kind="ExternalInput")
    db1b = nc.dram_tensor("db1b", [128, 4], F32, kind="ExternalInput")
    out = nc.dram_tensor("out", [NDEV, NCLS], BF16, kind="ExternalOutput")

    # global segment list (expert, lo, hi), then intersect with each 512 tile
    segs = []
    prev = 0
    for e, b in enumerate(bounds):
        if b > prev:
            segs.append((e, prev, b))
            prev = b
    tile_segs = []
    for t in range(NT):
        t0, t1 = t * TT, (t + 1) * TT
        tile_segs.append(
            [(e, max(lo, t0) - t0, min(hi, t1) - t0) for (e, lo, hi) in segs
             if lo < t1 and hi > t0]
        )

    with tile.TileContext(nc) as tc, ExitStack() as ctx:
        singles = ctx.enter_context(tc.tile_pool(name="singles", bufs=1))
        flatP = ctx.enter_context(tc.tile_pool(name="flatP", bufs=48))
        htP = ctx.enter_context(tc.tile_pool(name="htP", bufs=6))
        selP = ctx.enter_context(tc.tile_pool(name="selP", bufs=6))
        sigP = ctx.enter_context(tc.tile_pool(name="sigP", bufs=6))
        outP = ctx.enter_context(tc.tile_pool(name="outP", bufs=6))

        psA = ctx.enter_context(tc.tile_pool(name="psA", bufs=2, space="PSUM"))
        psB = ctx.enter_context(tc.tile_pool(name="psB", bufs=4, space="PSUM"))
        psC = ctx.enter_context(tc.tile_pool(name="psC", bufs=2, space="PSUM"))

        flat_tiles = [None] * NT

        def load_tile(t):
            need = sorted({c for (e, _, _) in tile_segs[t]
                           for c in range(KLO[e], KLO[e] + NK[e])})
            d = {}
            for c in need:
                fc = flatP.tile([128, TT], BF16, name="fc")
                nc.sync.dma_start(
                    out=fc,
                    in_=bass.AP(tensor=fusionT, offset=c * 128 * NDEV + t * TT,
                                ap=[[NDEV, 128], [1, TT]]),
                )
                d[c] = fc
            flat_tiles[t] = d

        # DMA rings: sync carries W1 weights + activation tiles (the critical
        # path), scalar carries the remaining weights and all output writes.
        # Order = order of first use so compute starts ASAP.
        expert_order = []
        for segl in tile_segs:
            for (e, _, _) in segl:
                if e not in expert_order:
                    expert_order.append(e)
        w1sb = [singles.tile([128, NK[e] * 512], BF16, name=f"w1sb{e}")
                for e in range(NE)]
        e0_ = expert_order[0]
        half = NK[e0_] * 512 // 2
        nc.sync.dma_start(out=w1sb[e0_][:, :half], in_=w1s[e0_][:, :half])
        b1sb = singles.tile([128, NE * 4], F32)
        nc.scalar.dma_start(out=b1sb, in_=b1b[:, :])
        b2sb = singles.tile([128, NE * 4], F32)
        nc.scalar.dma_start(out=b2sb, in_=b2b[:, :])
        db1sb = singles.tile([128, 4], F32)
        nc.scalar.dma_start(out=db1sb, in_=db1b[:, :])
        load_tile(0)
        nc.sync.dma_start(out=w1sb[e0_][:, half:], in_=w1s[e0_][:, half:])
        w2sb = singles.tile([128, NE * 2048], BF16)
        nc.scalar.dma_start(out=w2sb, in_=w2s[:, :])
        dw1sb = singles.tile([128, 2048], BF16)
        nc.scalar.dma_start(out=dw1sb, in_=dw1s[:, :])
        for e in expert_order[1:]:
            nc.sync.dma_start(out=w1sb[e], in_=w1s[e][:, :])
        dw2sb = singles.tile([128, 4 * NCLS], BF16)
        nc.scalar.dma_start(out=dw2sb, in_=dw2s[:, :])
        for e in range(NE):
            if e not in expert_order:
                nc.sync.dma_start(out=w1sb[e], in_=w1s[e][:, :])

        sig_tiles = [None] * NT

        def emit_front(t):
            """W1 -> relu -> W2 -> (+b2) -> selT for tile t."""
            ft = flat_tiles[t]
            selB = [psB.tile([128, TT], F32, name="selps") for _ in range(4)]
            nseg = len(tile_segs[t])
            for si, (e, lo, hi) in enumerate(tile_segs[t]):
                w = hi - lo
                for m in range(4):
                    hps = psA.tile([128, TT], F32, name="hps")
                    for ki in range(NK[e]):
                        nc.tensor.matmul(
                            hps[:, :w],
                            w1sb[e][:, (ki * 4 + m) * 128: (ki * 4 + m + 1) * 128],
                            ft[KLO[e] + ki][:, lo:hi],
                            start=(ki == 0),
                            stop=(ki == NK[e] - 1),
                        )
                    ht = htP.tile([128, TT], BF16, name="ht")
                    nc.scalar.activation(
                        ht[:, :w], hps[:, :w], AF.Relu,
                        bias=b1sb[:, e * 4 + m: e * 4 + m + 1], scale=1.0,
                    )
                    for md in range(4):
                        nc.tensor.matmul(
                            selB[md][:, lo:hi],
                            w2sb[:, e * 2048 + m * 512 + md * 128:
                                 e * 2048 + m * 512 + (md + 1) * 128],
                            ht[:, :w],
                            start=(m == 0),
                            stop=(m == 3),
                        )
            selT = []
            for md in range(4):
                st = selP.tile([128, TT], BF16, name="st")
                for (e, lo, hi) in tile_segs[t]:
                    nc.scalar.activation(
                        st[:, lo:hi], selB[md][:, lo:hi], AF.Identity,
                        bias=b2sb[:, e * 4 + md: e * 4 + md + 1], scale=1.0,
                    )
                selT.append(st)
            return selT

        def emit_dec1(t, selT):
            sigT = []
            for mh in range(4):
                ps = psA.tile([128, TT], F32, name="hps")
                for kd in range(4):
                    nc.tensor.matmul(
                        ps,
                        dw1sb[:, kd * 512 + mh * 128: kd * 512 + (mh + 1) * 128],
                        selT[kd],
                        start=(kd == 0),
                        stop=(kd == 3),
                    )
                sg = sigP.tile([128, TT], BF16, name="sg")
                nc.scalar.activation(
                    sg, ps, AF.Sigmoid, bias=db1sb[:, mh: mh + 1], scale=1.0,
                )
                sigT.append(sg)
            sig_tiles[t] = sigT

        def emit_dec2(t):
            sigT = sig_tiles[t]
            for n in range(NCH):
                nw = min(512, NCLS - n * 512)
                for s in range(TT // 128):
                    ps = psC.tile([128, 512], F32, name="d2ps")
                    for kh in range(4):
                        nc.tensor.matmul(
                            ps[:, :nw],
                            sigT[kh][:, s * 128: (s + 1) * 128],
                            dw2sb[:, kh * NCLS + n * 512: kh * NCLS + n * 512 + nw],
                            start=(kh == 0),
                            stop=(kh == 3),
                        )
                    ot = outP.tile([128, 512], BF16, name="ot")
                    nc.vector.tensor_copy(out=ot[:, :nw], in_=ps[:, :nw])
                    nc.scalar.dma_start(
                        out=out[t * TT + s * 128: t * TT + (s + 1) * 128,
                                n * 512: n * 512 + nw],
                        in_=ot[:, :nw],
                    )

        # software pipeline: dec2 of tile t-1 (a long PE stretch with no new
        # ACT dependencies) is emitted between W2(t) and dec1(t) so the PE
        # never waits on the ACT copies feeding dec1/dec2.
        for t in range(NT):
            if t + 1 < NT:
                load_tile(t + 1)
            selT = emit_front(t)
            if t > 0:
                emit_dec2(t - 1)
            emit_dec1(t, selT)
        emit_dec2(NT - 1)

    import bass_rust

    bass_rust.generate_event_semaphores(nc)
    return nc


_NC_CACHE = {}


def _get_nc(bounds):
    key = tuple(bounds)
    if key not in _NC_CACHE:
        _NC_CACHE[key] = _build(key)
    return _NC_CACHE[key]


def _swizzle(w):
    """[K, M] f32 -> lhsT SBUF layout [128, (K/128)*(M/128)*128] bf16 where
    col (k*(M/128)+m)*128+j holds w[k*128+p, m*128+j]."""
    K, M = w.shape
    return np.ascontiguousarray(
        w.reshape(K // 128, 128, M // 128, 128).transpose(1, 0, 2, 3)
        .reshape(128, (K // 128) * M).astype(bf16)
    )


def _prep(inputs):
    f32 = np.float32
    x = np.asarray(inputs["fusion_hs"], f32)                 # [L, B, D]
    flat = np.ascontiguousarray(np.transpose(x, (1, 0, 2)).reshape(B, L * D))

    logits = flat.astype(np.float64) @ np.asarray(inputs["gate_W"], f32).astype(
        np.float64
    ) + np.asarray(inputs["gate_b"], f32).astype(np.float64)
    am = np.argmax(logits, axis=1)

    idx = [np.nonzero(am == e)[0] for e in range(NE)]
    # per-core per-expert slot counts: multiples of 128 summing to NDEV,
    # largest-remainder so tile boundaries mostly align with expert
    # boundaries (pure single-expert tiles -> no split matmul groups).
    # Tokens that don't fit their expert's slots run on the host instead.
    want = [len(idx[e]) / NCORES / 128 for e in range(NE)]
    ke = [int(w) for w in want]
    while sum(ke) < NDEV // 128:
        ke[max(range(NE), key=lambda e: want[e] - ke[e])] += 1
    ke = [k * 128 for k in ke]
    ke_dev = [min(ke[e], len(idx[e]) // NCORES) for e in range(NE)]
    pad = NDEV - sum(ke)
    assert pad == 0
    bounds = (ke[0], ke[0] + ke[1], ke[0] + ke[1] + ke[2], NDEV)

    w1_3s = np.array(inputs["e3_W1"], f32, copy=True)
    w1_3s[: 3 * D] *= f32(np.asarray(inputs["e3_a"]).reshape(-1)[0])
    w1_3s[3 * D:] *= f32(np.asarray(inputs["e3_b"]).reshape(-1)[0])
    W1 = [np.asarray(inputs["e0_W1"], f32), np.asarray(inputs["e1_W1"], f32),
          np.asarray(inputs["e2_W1"], f32), w1_3s]
    W2 = [np.asarray(inputs[f"e{e}_W2"], f32) for e in range(NE)]
    b1 = [np.asarray(inputs[f"e{e}_b1"], f32) for e in range(NE)]
    b2 = [np.asarray(inputs[f"e{e}_b2"], f32) for e in range(NE)]
    dW1 = np.asarray(inputs["dec_W1"], f32)
    db1 = np.asarray(inputs["dec_b1"], f32)
    dW2 = np.asarray(inputs["dec_W2"], f32)
    db2 = np.asarray(inputs["dec_b2"], f32)

    common = {
        "w1s0": _swizzle(W1[0]), "w1s1": _swizzle(W1[1]),
        "w1s2": _swizzle(W1[2]), "w1s3": _swizzle(W1[3]),
        "w2s": np.concatenate([_swizzle(w) for w in W2], axis=1),
        "dw1s": _swizzle(dW1),
        "dw2s": np.ascontiguousarray(
            dW2.reshape(4, 128, NCLS).transpose(1, 0, 2).reshape(128, 4 * NCLS)
            .astype(bf16)
        ),
        "b1b": np.stack([b1[e][m * 128: (m + 1) * 128]
                         for e in range(NE) for m in range(4)], axis=1),
        "b2b": np.stack([b2[e][m * 128: (m + 1) * 128]
                         for e in range(NE) for m in range(4)], axis=1),
        "db1b": np.stack([db1[m * 128: (m + 1) * 128] for m in range(4)], axis=1),
    }

    perms, in_maps = [], []
    for c in range(NCORES):
        src = np.full(NDEV, -1, np.int64)
        base = 0
        for e in range(NE):
            k = ke_dev[e]
            src[base: base + k] = idx[e][c * k: (c + 1) * k]
            base += ke[e]
        perms.append(src)
        valid = src >= 0
        a = np.zeros((NDEV, L * D), bf16)
        a[valid] = flat[src[valid]].astype(bf16)
        m = dict(common)
        m["fusionT"] = np.ascontiguousarray(a.T)
        in_maps.append(m)

    # overflow tokens: full forward on host in fp32
    lt = np.concatenate([idx[e][NCORES * ke_dev[e]:] for e in range(NE)])
    lt_out = np.zeros((len(lt), NCLS), f32)
    if len(lt):
        off = 0
        ins = [flat[:, : 3 * D], flat[:, 3 * D:], flat, flat]
        for e in range(NE):
            g = idx[e][NCORES * ke_dev[e]:]
            if len(g) == 0:
                continue
            h = np.maximum(ins[e][g] @ W1[e] + b1[e], 0)
            sel = h @ W2[e] + b2[e]
            sig = 1.0 / (1.0 + np.exp(-(sel @ dW1 + db1)))
            lt_out[off: off + len(g)] = sig @ dW2 + db2
            off += len(g)

    return bounds, in_maps, perms, lt, lt_out, db2


def kernel(**inputs):
    bounds, in_maps, perms, lt, lt_out, db2 = _prep(inputs)
    nc = _get_nc(bounds)
    res = run_bass_kernel_spmd(nc, in_maps, core_ids=list(range(NCORES)))
    out = np.empty((B, NCLS), np.float32)
    for c in range(NCORES):
        dev = np.asarray(res.results[c]["out"], np.float32)
        valid = perms[c] >= 0
        out[perms[c][valid]] = dev[valid] + db2
    if len(lt):
        out[lt] = lt_out
    return out
